# revision 3
# baseline (speedup 1.0000x reference)
"""Trainium2 Bass kernel for nn_ActQuantWrapper (hadamard + per-token act quant + linear).

Math (per reference):
  z = (H_64 (x) kron I_had) x / 8           -- FHT over 64 groups along feature dim
  sx[t] = clip(absmax(z[t,:])/127, 1e-5)    -- per-token scale
  xq = round(z/sx)*sx                        -- act quant-dequant
  out = xq @ weight.T + bias                 -- weight already per-channel quantized

Device strategy (8 cores, data-parallel over tokens, weight replicated):
  - qx = round(z/sx) and qw = round(w/sw) are integers in [-127,127]: exactly
    representable in bf16, so the 4096^2 x 512 matmul runs at full bf16 PE rate
    and the result is scaled by sx[t]*sw[o] afterward (near-exact numerics).
  - The weight arrives already quantized, so bf16(w * (1/sw)) lands exactly on
    the integer grid without explicit rounding.
  - Activation rounding uses the fp32 magic-number trick (+1.5*2^23, -1.5*2^23).
  - bf16 tensors are transposed k-major via DMA xbar transpose.
"""

import math

import numpy as np

import concourse.bass as bass
import concourse.tile as tile
from concourse import bacc, mybir
from concourse.bass_utils import run_bass_kernel_spmd

F32 = mybir.dt.float32
BF16 = mybir.dt.bfloat16
MAGIC = 12582912.0  # 1.5 * 2**23: adding then subtracting rounds f32 to int (RNE)

N_CORES = 8
B, S, D_IN, D_OUT = 2, 2048, 4096, 4096
N_TOK = B * S
T_CORE = N_TOK // N_CORES  # 512 tokens per core
N_GROUPS = 64              # hadamard dimension (fixed by reference)


def build_kernel(n_tok, K, O, oc_size, trace_sim=False):
    """Build + compile the per-core kernel.

    n_tok: tokens per core (multiple of 128)
    K:     in features  (N_GROUPS * had_dim, multiple of 128)
    O:     out features (multiple of oc_size)
    oc_size: output-chunk width for the matmul (multiple of 128, <= 512)
    """
    assert n_tok % 128 == 0 and K % 128 == 0 and O % oc_size == 0
    assert oc_size % 128 == 0
    n_tt = n_tok // 128     # token tiles
    n_kt = K // 128         # contraction tiles
    n_oc = O // oc_size     # output chunks
    ot_per_oc = oc_size // 128
    had_dim = K // N_GROUPS

    nc = bacc.Bacc("TRN2", target_bir_lowering=False, debug=False)
    x_d = nc.dram_tensor("x", [n_tok, K], F32, kind="ExternalInput")
    w_d = nc.dram_tensor("w", [O, K], F32, kind="ExternalInput")
    b_d = nc.dram_tensor("b", [O], F32, kind="ExternalInput")
    out_d = nc.dram_tensor("out", [n_tok, O], F32, kind="ExternalOutput")
    swsc_d = nc.dram_tensor("swsc", [O], F32)  # internal scratch for sw broadcast

    with tile.TileContext(nc, trace_sim=trace_sim) as tc:
        with (
            tc.tile_pool(name="xload", bufs=2) as xload,
            tc.tile_pool(name="xwork", bufs=1) as xwork,
            tc.tile_pool(name="qxp", bufs=1) as qxp,
            tc.tile_pool(name="qxT", bufs=1) as qxTp,
            tc.tile_pool(name="wload", bufs=2) as wload,
            tc.tile_pool(name="wq", bufs=1) as wqp,
            tc.tile_pool(name="qwT", bufs=2) as qwTp,
            tc.tile_pool(name="bcast", bufs=2) as bcast,
            tc.tile_pool(name="outp", bufs=2) as outp,
            tc.tile_pool(name="small", bufs=2) as small,
            tc.tile_pool(name="consts", bufs=1) as consts,
            tc.tile_pool(name="psum", bufs=2, space=bass.MemorySpace.PSUM) as psum,
        ):
            qxT = qxTp.tile([128, n_kt, n_tok], BF16)
            sx_all = consts.tile([128, n_tt], F32)

            # ---------------- x path: FHT -> quant -> transpose ----------------
            for tt in range(n_tt):
                za = xload.tile([128, K], F32, tag="za")
                nc.sync.dma_start(za[:], x_d.ap()[tt * 128:(tt + 1) * 128, :])
                zb = xwork.tile([128, K], F32, tag="zb")
                bufs = [za, zb]
                for s in range(6):
                    src, dst = bufs[s % 2], bufs[(s + 1) % 2]
                    blk = had_dim << s
                    sv = src[:].rearrange("p (a c b) -> p a c b", c=2, b=blk)
                    dv = dst[:].rearrange("p (a c b) -> p a c b", c=2, b=blk)
                    nc.vector.tensor_add(dv[:, :, 0, :], sv[:, :, 0, :], sv[:, :, 1, :])
                    nc.vector.tensor_sub(dv[:, :, 1, :], sv[:, :, 0, :], sv[:, :, 1, :])
                # 6 stages end back in za (unscaled by 1/8; folded into the scale)
                m = small.tile([128, 1], F32, tag="xm")
                nc.vector.tensor_reduce(
                    out=m[:], in_=za[:], axis=mybir.AxisListType.X,
                    op=mybir.AluOpType.max, apply_absolute_value=True,
                )
                # sx = clip((m/8)/127, 1e-5) = clip(m/1016, 1e-5); m/8 is exact
                nc.vector.tensor_scalar(
                    out=sx_all[:, tt:tt + 1], in0=m[:],
                    scalar1=float(np.float32(1.0) / np.float32(1016.0)),
                    scalar2=1e-5,
                    op0=mybir.AluOpType.mult, op1=mybir.AluOpType.max,
                )
                rx = small.tile([128, 1], F32, tag="xr")
                nc.vector.reciprocal(rx[:], sx_all[:, tt:tt + 1])
                # q = round(z/8 / sx) = round(z * (rx/8)); do rx/8 on the tiny tile
                rx8 = small.tile([128, 1], F32, tag="xr8")
                nc.vector.tensor_scalar_mul(rx8[:], rx[:], 0.125)
                qtmp = xwork.tile([128, K], F32, tag="zb")
                nc.scalar.activation(
                    out=qtmp[:], in_=za[:], func=mybir.ActivationFunctionType.Copy,
                    bias=MAGIC, scale=rx8[:],
                )
                qx = qxp.tile([128, K], BF16, tag="qx")
                nc.vector.tensor_scalar_add(qx[:], qtmp[:], -MAGIC)
                nc.scalar.dma_start_transpose(
                    qxT[:, :, tt * 128:(tt + 1) * 128], qx[:]
                )

            # ---------------- weight path + matmul, per output chunk ----------------
            for oc in range(n_oc):
                qwT = qwTp.tile([128, n_kt, oc_size], BF16, tag="qwT")
                for j in range(ot_per_oc):
                    ot = oc * ot_per_oc + j
                    wt = wload.tile([128, K], F32, tag="wt")
                    nc.sync.dma_start(wt[:], w_d.ap()[ot * 128:(ot + 1) * 128, :])
                    wm = small.tile([128, 1], F32, tag="wm")
                    nc.vector.tensor_reduce(
                        out=wm[:], in_=wt[:], axis=mybir.AxisListType.X,
                        op=mybir.AluOpType.max, apply_absolute_value=True,
                    )
                    sw = small.tile([128, 1], F32, tag="sw")
                    nc.vector.tensor_scalar(
                        out=sw[:], in0=wm[:],
                        scalar1=float(np.float32(1.0) / np.float32(127.0)),
                        scalar2=1e-5,
                        op0=mybir.AluOpType.mult, op1=mybir.AluOpType.max,
                    )
                    nc.sync.dma_start(swsc_d.ap()[ot * 128:(ot + 1) * 128], sw[:])
                    rw = small.tile([128, 1], F32, tag="rw")
                    nc.vector.reciprocal(rw[:], sw[:])
                    # weight is pre-quantized: w*rw lands within 5e-5 of an integer,
                    # so the bf16 output conversion rounds exactly onto the grid.
                    qw = wqp.tile([128, K], BF16, tag="qw")
                    nc.scalar.activation(
                        out=qw[:], in_=wt[:], func=mybir.ActivationFunctionType.Copy,
                        bias=0.0, scale=rw[:],
                    )
                    nc.scalar.dma_start_transpose(
                        qwT[:, :, j * 128:(j + 1) * 128], qw[:]
                    )

                # sw/bias broadcast tiles for this chunk ([128, oc_size], partition bcast)
                swb = bcast.tile([128, oc_size], F32, tag="swb")
                src = swsc_d.ap()[oc * oc_size:(oc + 1) * oc_size]
                nc.gpsimd.dma_start(
                    out=swb[:],
                    in_=bass.AP(tensor=src.tensor, offset=src.offset,
                                ap=[[0, 128]] + list(src.ap)),
                )
                bb = bcast.tile([128, oc_size], F32, tag="bb")
                srcb = b_d.ap()[oc * oc_size:(oc + 1) * oc_size]
                nc.gpsimd.dma_start(
                    out=bb[:],
                    in_=bass.AP(tensor=srcb.tensor, offset=srcb.offset,
                                ap=[[0, 128]] + list(srcb.ap)),
                )

                for t in range(n_tt):
                    ps = psum.tile([128, oc_size], F32, tag=f"ps{t % 4}")
                    for k in range(n_kt):
                        nc.tensor.matmul(
                            ps[:],
                            qxT[:, k, t * 128:(t + 1) * 128],
                            qwT[:, k, :],
                            start=(k == 0), stop=(k == n_kt - 1),
                        )
                    o_sb = outp.tile([128, oc_size], F32, tag="osb")
                    # out = (psum * sx[t]) * swb  (+ bias below)
                    nc.vector.scalar_tensor_tensor(
                        out=o_sb[:], in0=ps[:], scalar=sx_all[:, t:t + 1], in1=swb[:],
                        op0=mybir.AluOpType.mult, op1=mybir.AluOpType.mult,
                    )
                    nc.vector.tensor_add(o_sb[:], o_sb[:], bb[:])
                    nc.sync.dma_start(
                        out_d.ap()[t * 128:(t + 1) * 128,
                                   oc * oc_size:(oc + 1) * oc_size],
                        o_sb[:],
                    )

    nc.compile()
    return nc


_CACHED = None


def _get_full_kernel():
    global _CACHED
    if _CACHED is None:
        _CACHED = build_kernel(T_CORE, D_IN, D_OUT, 256)
    return _CACHED


def kernel(x, weight, bias, had_dim):
    assert int(had_dim) == 64
    assert x.shape == (B, S, D_IN) and weight.shape == (D_OUT, D_IN)
    nc = _get_full_kernel()
    xf = np.ascontiguousarray(x.reshape(N_TOK, D_IN), dtype=np.float32)
    w = np.ascontiguousarray(weight, dtype=np.float32)
    bi = np.ascontiguousarray(bias, dtype=np.float32)
    in_maps = [
        {"x": xf[i * T_CORE:(i + 1) * T_CORE], "w": w, "b": bi}
        for i in range(N_CORES)
    ]
    res = run_bass_kernel_spmd(nc, in_maps, core_ids=list(range(N_CORES)))
    out = np.concatenate([r["out"] for r in res.results], axis=0)
    return out.reshape(B, S, D_OUT)


if __name__ == "__main__":
    # quick shape smoke
    rng = np.random.default_rng(0)
    x = rng.standard_normal((B, S, D_IN), dtype=np.float32)
    w = rng.standard_normal((D_OUT, D_IN), dtype=np.float32)
    b = rng.standard_normal(D_OUT).astype(np.float32)
    o = kernel(x, w, b, np.int64(64))
    print(o.shape, o.dtype)
